# revision 19
# baseline (speedup 1.0000x reference)
"""MLA attention kernel for TRN2, SPMD over 8 NeuronCores.

Sharding: core c = 4*b + g  (b = batch 0..1, g = head-group 0..3, 4 heads each).
Each core computes, for its batch b and head-group g:
    qT = (Wq_g*scale)^T x^T + bq_g*scale        [256, 2048]   (bf16)
    latT = Wl^T x^T + bl                        [256, 2048]
    kT = Wk_g^T latT                            [256, 2048]   (bk dropped: softmax shift-invariant)
    v  = latT^T Wv_g                            [2048, 256]   (bv folded into host const)
    per head h: sT = kT_h^T qT_h ; pT = exp(sT) (no max-subtraction; scores ~ N(0,1))
                oT = v_h^T pT ; L = 1^T pT ; aT = oT / L
    partial = A Wo_g                            [2048, 1024]  (f32)
Host sums the 4 partials per batch and adds (bv @ Wo + bo).

Emission is kchunk-granular: per k-chunk t the two QK matmuls (row-groups 0/64)
are adjacent for PE row-tile concurrency; PV/L matmuls are ordered
[PVh0, L_h1, PVh1, L_h0] with disjoint col-groups for the same reason.
"""
import contextlib
import ctypes
import os
import sys
import types

if "/opt/trn_rl_repo" not in sys.path:
    sys.path.insert(0, "/opt/trn_rl_repo")

import numpy as np
import ml_dtypes

NPBF16 = ml_dtypes.bfloat16
SCALE = 64 ** -0.5
_STATE = {}


# ---------------------------------------------------------------- ntff shim
def _install_ntff_shim():
    """Provide antenv.axon_hooks so run_bass_kernel_spmd(trace=True) works."""
    if "antenv.axon_hooks" in sys.modules:
        return
    try:
        import antenv
    except ImportError:
        return

    so_path = "/opt/axon/libaxon_pjrt.so"

    def _hook_factory():
        try:
            lib = ctypes.CDLL(so_path)
        except OSError:
            return None
        if not hasattr(lib, "axon_start_nrt_profile"):
            return None
        lib.axon_start_nrt_profile.argtypes = [ctypes.POINTER(ctypes.c_int64), ctypes.c_size_t]
        lib.axon_start_nrt_profile.restype = ctypes.c_int64
        lib.axon_stop_nrt_profile.argtypes = [ctypes.c_char_p]
        lib.axon_stop_nrt_profile.restype = ctypes.c_int64

        @contextlib.contextmanager
        def _hook(output_dir, device_ids):
            import jax

            jax.devices()
            if device_ids:
                ids = (ctypes.c_int64 * len(device_ids))(*device_ids)
                rc = lib.axon_start_nrt_profile(ids, len(device_ids))
            else:
                rc = lib.axon_start_nrt_profile(None, 0)
            if rc != 0:
                raise RuntimeError(f"axon_start_nrt_profile rc={rc}")
            try:
                yield
            finally:
                n = lib.axon_stop_nrt_profile(str(output_dir).encode())
                print(f"profile: {n} file(s) written to {output_dir}", file=sys.stderr)

        return _hook

    import antenv

    mod = types.ModuleType("antenv.axon_hooks")
    _state = {"hook": _hook_factory()}
    mod.set_axon_ntff_profile_hook = lambda h: _state.__setitem__("hook", h)
    mod.get_axon_ntff_profile_hook = lambda: _state["hook"]
    sys.modules["antenv.axon_hooks"] = mod
    antenv.axon_hooks = mod


# ---------------------------------------------------------------- bass build
def _build_nc(debug_dump=False):
    import concourse.bass as bass  # noqa: F401
    import concourse.tile as tile
    from concourse import bacc, mybir

    BF16 = mybir.dt.bfloat16
    F32 = mybir.dt.float32
    EXP = mybir.ActivationFunctionType.Exp
    CPY = mybir.ActivationFunctionType.Copy
    LN = mybir.ActivationFunctionType.Ln

    nc = bacc.Bacc(None, target_bir_lowering=False, debug=False)

    xT = nc.dram_tensor("xT", [128, 8, 2048], BF16, kind="ExternalInput")
    wq = nc.dram_tensor("wq", [128, 8, 256], BF16, kind="ExternalInput")
    bq = nc.dram_tensor("bq", [128, 2], F32, kind="ExternalInput")
    wl = nc.dram_tensor("wl", [128, 8, 256], BF16, kind="ExternalInput")
    bl = nc.dram_tensor("bl", [128, 2], F32, kind="ExternalInput")
    wk = nc.dram_tensor("wk", [128, 2, 256], BF16, kind="ExternalInput")
    wv = nc.dram_tensor("wv", [128, 2, 256], BF16, kind="ExternalInput")
    wo = nc.dram_tensor("wo", [128, 2, 1024], BF16, kind="ExternalInput")
    out = nc.dram_tensor("out", [2048, 1024], F32, kind="ExternalOutput")

    with nc.allow_low_precision("bf16 intermediates by design"), tile.TileContext(nc) as tc:
        with (
            tc.tile_pool(name="wpool", bufs=1) as wpool,
            tc.tile_pool(name="xpool", bufs=1) as xpool,
            tc.tile_pool(name="proj", bufs=1) as proj,
            tc.tile_pool(name="ptp", bufs=50) as ptp,
            tc.tile_pool(name="atp", bufs=4) as atp,
            tc.tile_pool(name="obp", bufs=4) as obp,
            tc.tile_pool(name="rpool", bufs=2) as rpool,
            tc.tile_pool(name="ps", bufs=2, space="PSUM") as ps,
        ):
            # ---------------- constants + inputs
            x_kn = [
                [xpool.tile([128, 512], BF16, name=f"x_{k}_{n}") for n in range(4)]
                for k in range(8)
            ]
            wq_sb = wpool.tile([128, 8, 256], BF16)
            wl_sb = wpool.tile([128, 8, 256], BF16)
            wk_sb = wpool.tile([128, 2, 256], BF16)
            wv_sb = wpool.tile([128, 2, 256], BF16)
            wo_sb = wpool.tile([128, 2, 1024], BF16)
            bq_sb = wpool.tile([128, 2], F32)
            bl_sb = wpool.tile([128, 2], F32)
            ones_k1 = wpool.tile([128, 64], BF16)
            ones_sb = wpool.tile([128, 1], BF16)
            nc.vector.memset(ones_k1[:], 1.0)
            nc.vector.memset(ones_sb[:], 1.0)

            nc.sync.dma_start(out=wq_sb[:], in_=wq[:])
            nc.sync.dma_start(out=bq_sb[:], in_=bq[:])
            nc.sync.dma_start(out=wl_sb[:], in_=wl[:])
            nc.sync.dma_start(out=bl_sb[:], in_=bl[:])
            nc.sync.dma_start(out=wk_sb[:], in_=wk[:])
            for n in range(4):
                for k in range(8):
                    nc.sync.dma_start(
                        out=x_kn[k][n][:],
                        in_=xT[:, k, 512 * n : 512 * n + 512],
                    )
            nc.sync.dma_start(out=wv_sb[:], in_=wv[:])
            nc.sync.dma_start(out=wo_sb[:], in_=wo[:])

            latT_n = [proj.tile([128, 2, 512], BF16, name=f"latT_{i}") for i in range(4)]
            qT_n = [proj.tile([128, 2, 512], BF16, name=f"qT_{i}") for i in range(4)]
            kT_n = [proj.tile([128, 2, 512], BF16, name=f"kT_{i}") for i in range(4)]
            v_sb = proj.tile([128, 16, 256], BF16)

            def misc_ps(name):
                return ps.tile([128, 512], F32, tag="s", name=name, bufs=3)

            def ot_ps(name):
                return ps.tile([128, 512], F32, tag="ot", name=name, bufs=2)

            # HAM warm-up: dummy matmuls while input DMA is in flight
            warm_sb = wpool.tile([128, 512], BF16)
            nc.vector.memset(warm_sb[:], 0.25)
            warm_ps = misc_ps("warm_ps")
            for i in range(40):
                nc.tensor.matmul(
                    warm_ps[:], warm_sb[:, 0:128], warm_sb[:],
                    start=(i == 0), stop=(i == 39),
                )

            # ---------------- projection emitters (interleaved as fillers)
            def emit_lat(n, m):
                acc = misc_ps(f"lat_ps_{m}_{n}")
                for k in range(8):
                    nc.tensor.matmul(
                        acc[:],
                        wl_sb[:, k, 128 * m : 128 * m + 128],
                        x_kn[k][n][:],
                        start=(k == 0),
                        stop=(k == 7),
                    )
                nc.vector.tensor_scalar_add(
                    out=latT_n[n][:, m, :], in0=acc[:], scalar1=bl_sb[:, m : m + 1]
                )

            def emit_kt(n):
                for m in range(2):
                    acc = misc_ps(f"kt_ps_{m}_{n}")
                    for k in range(2):
                        nc.tensor.matmul(
                            acc[:],
                            wk_sb[:, k, 128 * m : 128 * m + 128],
                            latT_n[n][:, k, :],
                            start=(k == 0),
                            stop=(k == 1),
                        )
                    nc.vector.tensor_copy(out=kT_n[n][:, m, :], in_=acc[:])

            def emit_v(ts):
                for t in ts:
                    acc = misc_ps(f"v_ps_{t}")
                    for k in range(2):
                        nc.tensor.matmul(
                            acc[:, 0:256],
                            latT_n[t // 4][:, k, 128 * (t % 4) : 128 * (t % 4) + 128],
                            wv_sb[:, k, :],
                            start=(k == 0),
                            stop=(k == 1),
                        )
                    nc.vector.tensor_copy(out=v_sb[:, t, :], in_=acc[:, 0:256])

            def emit_qt(n, m):
                acc = misc_ps(f"q_ps_{m}_{n}")
                for k in range(8):
                    nc.tensor.matmul(
                        acc[:],
                        wq_sb[:, k, 128 * m : 128 * m + 128],
                        x_kn[k][n][:],
                        start=(k == 0),
                        stop=(k == 7),
                    )
                nc.vector.tensor_scalar_add(
                    out=qT_n[n][:, m, :], in0=acc[:], scalar1=bq_sb[:, m : m + 1]
                )

            # ---------------- attention phase machinery
            # L psum row per (pair, head-in-pair), chosen so each pvl-adjacent
            # matmul pair has disjoint PE col-groups (concurrency):
            #   pair0: PVh0(cols 0:64) | L_h1@96 ; PVh1(64:128) | L_h0@32
            #   pair1: PVh0 | L_h1@64 ; PVh1 | L_h0@0
            L_ROW = {0: (32, 96), 1: (0, 64)}
            PD = {}
            LT = {}

            def emit_phase(ic, p, fillers):
                """QK + exp for pair (ic,p), gi-granular (2 kchunks per gi).

                Per gi: 4 adjacent QK matmuls (row-groups alternate 0/64 ->
                pairwise PE row-tile concurrency), 2 exps, then fillers[gi].
                PD[key]["pt"][gi] = (pt_h0, pt_h1), each [128, 2(kchunk), 512].
                """
                key = (ic, p)
                PD[key] = {"pt": []}
                qTc = qT_n[ic]
                for gi in range(8):
                    s0 = ps.tile([128, 2, 512], F32, tag="s", name=f"s0_{ic}_{p}_{gi}", bufs=3)
                    s1 = ps.tile([128, 2, 512], F32, tag="s", name=f"s1_{ic}_{p}_{gi}", bufs=3)
                    for tt in range(2):
                        t = 2 * gi + tt
                        kTc = kT_n[t // 4]
                        ksl = slice(128 * (t % 4), 128 * (t % 4) + 128)
                        nc.tensor.matmul(
                            s0[:, tt, :], kTc[0:64, p, ksl], qTc[0:64, p, :],
                            start=True, stop=True,
                        )
                        nc.tensor.matmul(
                            s1[:, tt, :], kTc[64:128, p, ksl], qTc[64:128, p, :],
                            start=True, stop=True,
                        )
                    pt0 = ptp.tile([128, 2, 512], BF16, tag="pt", name=f"pt0_{ic}_{p}_{gi}")
                    pt1 = ptp.tile([128, 2, 512], BF16, tag="pt", name=f"pt1_{ic}_{p}_{gi}")
                    nc.scalar.activation(pt0[:], s0[:], EXP)
                    nc.scalar.activation(pt1[:], s1[:], EXP)
                    PD[key]["pt"].append((pt0, pt1))
                    for f in fillers.get(gi, ()):
                        f()

            def _pt(key, t, j):
                return PD[key]["pt"][t // 2][j][:, t % 2, :]

            def emit_pv_batch(key, gi):
                """8 PV matmuls (t = 4gi..4gi+3), all (128,64)-geometry."""
                d = PD[key]
                ic, p = key
                if gi == 0:
                    d["ot"] = ot_ps(f"ot_{ic}_{p}")
                h0, h1 = 2 * p, 2 * p + 1
                for t in range(4 * gi, 4 * gi + 4):
                    st, sp = (t == 0), (t == 15)
                    nc.tensor.matmul(
                        d["ot"][0:64, :], v_sb[:, t, 64 * h0 : 64 * h0 + 64],
                        _pt(key, t, 0),
                        start=st, stop=sp, skip_group_check=True,
                    )
                    nc.tensor.matmul(
                        d["ot"][64:128, :], v_sb[:, t, 64 * h1 : 64 * h1 + 64],
                        _pt(key, t, 1),
                        start=st, stop=sp, skip_group_check=True,
                    )

            def emit_l_quad(ic, qi):
                """16 L matmuls (both pairs, t = 4qi..4qi+3): 4-way col-tile quads."""
                if qi == 0:
                    LT[ic] = misc_ps(f"L_{ic}")
                    nc.vector.memset(LT[ic][:], 1.0)
                Lt = LT[ic]
                for t in range(4 * qi, 4 * qi + 4):
                    st, sp = (t == 0), (t == 15)
                    for p in range(2):
                        r0, r1 = L_ROW[p]
                        nc.tensor.matmul(
                            Lt[r0 : r0 + 1, :], ones_sb[:], _pt((ic, p), t, 0),
                            start=st, stop=sp, tile_position=(0, r0), skip_group_check=True,
                        )
                        nc.tensor.matmul(
                            Lt[r1 : r1 + 1, :], ones_sb[:], _pt((ic, p), t, 1),
                            start=st, stop=sp, tile_position=(0, r1), skip_group_check=True,
                        )

            RC = {}

            def emit_norm_recip(ic):
                """DVE recip of the ic's 4 L rows (emit at end of prev phase)."""
                Lt = LT.pop(ic)
                recip = rpool.tile([128, 512], BF16, tag="recip", name=f"recip_{ic}", bufs=2)
                nc.vector.reciprocal(out=recip[:], in_=Lt[:])
                RC[ic] = recip

            def emit_norm(ic):
                """PE broadcast of recip + DVE mult -> at tiles."""
                recip = RC.pop(ic)
                for p in range(2):
                    d = PD[(ic, p)]
                    bc_ps = misc_ps(f"bcp_{ic}_{p}")
                    for j in range(2):
                        row = L_ROW[p][j]
                        nc.tensor.matmul(
                            bc_ps[64 * j : 64 * j + 64, :],
                            ones_k1[row : row + 1, 0:64],
                            recip[row : row + 1, :],
                            start=True, stop=True,
                            tile_position=(row, 64 * j),
                            skip_group_check=True,
                        )
                    bc = rpool.tile([128, 512], F32, tag="bc", name=f"bcs_{ic}_{p}", bufs=2)
                    nc.vector.tensor_copy(out=bc[:], in_=bc_ps[:])
                    at = atp.tile([128, 512], BF16, tag="at", name=f"at_{ic}_{p}", bufs=4)
                    nc.vector.tensor_mul(out=at[0:64, :], in0=d["ot"][0:64, :], in1=bc[0:64, :])
                    nc.vector.tensor_mul(out=at[64:128, :], in0=d["ot"][64:128, :], in1=bc[64:128, :])
                    d["at"] = at

            def emit_wo_chunk(ic, u):
                at0 = PD[(ic, 0)]["at"]
                at1 = PD[(ic, 1)]["at"]
                for n2 in range(2):
                    wo_ps = ot_ps(f"wo_{ic}_{u}_{n2}")
                    for p, atx in ((0, at0), (1, at1)):
                        nc.tensor.matmul(
                            wo_ps[:],
                            atx[:, 128 * u : 128 * u + 128],
                            wo_sb[:, p, 512 * n2 : 512 * n2 + 512],
                            start=(p == 0),
                            stop=(p == 1),
                        )
                    ob = obp.tile([128, 512], F32, tag="ob", name=f"ob_{ic}_{u}_{n2}")
                    nc.vector.tensor_copy(out=ob[:], in_=wo_ps[:])
                    r0 = 512 * ic + 128 * u
                    nc.sync.dma_start(
                        out=out[r0 : r0 + 128, 512 * n2 : 512 * n2 + 512],
                        in_=ob[:],
                    )

            # ---------------- schedule
            emit_qt(0, 0)
            emit_qt(0, 1)
            emit_lat(0, 0)
            emit_lat(0, 1)
            emit_kt(0)

            def fillers_pv0(ic, extra=None):
                """Phase emit_phase(ic,1): PV batches of pair (ic,0) at gi 5-7."""
                f = {gi: [] for gi in range(8)}
                if extra:
                    for gi, fns in extra.items():
                        f[gi].extend(fns)
                f[5].append(lambda: emit_pv_batch((ic, 0), 0))
                f[6].append(lambda: emit_pv_batch((ic, 0), 1))
                f[7].append(lambda: emit_pv_batch((ic, 0), 2))
                f[7].append(lambda: emit_pv_batch((ic, 0), 3))
                return f

            def fillers_pv1(ic, extra=None):
                """Next phase: L quads of ic (both pairs) + recip + PV of (ic,1)."""
                f = {gi: [] for gi in range(8)}
                if extra:
                    for gi, fns in extra.items():
                        f[gi].extend(fns)
                for qi in range(4):
                    f[1 + qi].append(lambda qi=qi: emit_l_quad(ic, qi))
                f[4].append(lambda: emit_norm_recip(ic))
                f[5].append(lambda: emit_pv_batch((ic, 1), 0))
                f[6].append(lambda: emit_pv_batch((ic, 1), 1))
                f[7].append(lambda: emit_pv_batch((ic, 1), 2))
                f[7].append(lambda: emit_pv_batch((ic, 1), 3))
                return f

            # P0: pair (0,0) — projections as fillers (gi-keyed)
            emit_phase(0, 0, {
                0: [lambda: emit_lat(1, 0)],
                1: [lambda: emit_lat(1, 1), lambda: emit_kt(1)],
                2: [lambda: emit_lat(2, 0)],
                3: [lambda: emit_lat(2, 1), lambda: emit_kt(2)],
                4: [lambda: emit_lat(3, 0), lambda: emit_v(range(0, 4))],
                5: [lambda: emit_lat(3, 1), lambda: emit_kt(3)],
                6: [lambda: emit_v(range(4, 8)), lambda: emit_v(range(8, 12))],
                7: [lambda: emit_v(range(12, 16)), lambda: emit_qt(1, 0)],
            })
            # P1: pair (0,1)
            emit_phase(0, 1, fillers_pv0(0, extra={
                0: [lambda: emit_qt(1, 1)],
                1: [lambda: emit_qt(2, 0)],
                2: [lambda: emit_qt(2, 1)],
            }))
            # P2: pair (1,0)
            emit_phase(1, 0, fillers_pv1(0, extra={
                0: [lambda: emit_qt(3, 0), lambda: emit_qt(3, 1)],
            }))
            # P3: pair (1,1)
            emit_phase(1, 1, fillers_pv0(1, extra={
                0: [lambda: emit_norm(0)],
                1: [lambda: emit_wo_chunk(0, 0), lambda: emit_wo_chunk(0, 1)],
                2: [lambda: emit_wo_chunk(0, 2)],
                3: [lambda: emit_wo_chunk(0, 3)],
            }))
            # P4: pair (2,0)
            emit_phase(2, 0, fillers_pv1(1))
            # P5: pair (2,1)
            emit_phase(2, 1, fillers_pv0(2, extra={
                0: [lambda: emit_norm(1)],
                1: [lambda: emit_wo_chunk(1, 0), lambda: emit_wo_chunk(1, 1)],
                2: [lambda: emit_wo_chunk(1, 2)],
                3: [lambda: emit_wo_chunk(1, 3)],
            }))
            # P6: pair (3,0)
            emit_phase(3, 0, fillers_pv1(2))
            # P7: pair (3,1)
            emit_phase(3, 1, fillers_pv0(3, extra={
                0: [lambda: emit_norm(2)],
                1: [lambda: emit_wo_chunk(2, 0), lambda: emit_wo_chunk(2, 1)],
                2: [lambda: emit_wo_chunk(2, 2)],
                3: [lambda: emit_wo_chunk(2, 3)],
            }))
            # tail
            for gi in range(4):
                emit_pv_batch((3, 1), gi)
            for qi in range(4):
                emit_l_quad(3, qi)
            emit_norm_recip(3)
            emit_norm(3)
            for u in range(4):
                emit_wo_chunk(3, u)

    nc.compile()
    return nc


def _get_nc():
    if "nc" not in _STATE:
        _STATE["nc"] = _build_nc()
    return _STATE["nc"]


# ---------------------------------------------------------------- host side
def _pack_k(a, kchunks):
    """[K, N] f32/bf16 -> [128, kchunks, N] bf16 (K = 128*kchunks)."""
    K, N = a.shape
    return np.ascontiguousarray(
        np.asarray(a, np.float32).reshape(kchunks, 128, N).transpose(1, 0, 2)
    ).astype(NPBF16)


def kernel(x, Wq, bq, Wl, bl, Wk, bk, Wv, bv, Wo, bo):
    x = np.asarray(x, np.float32)
    Wq = np.asarray(Wq, np.float32)
    bq = np.asarray(bq, np.float32)
    Wl = np.asarray(Wl, np.float32)
    bl = np.asarray(bl, np.float32)
    Wk = np.asarray(Wk, np.float32)
    Wv = np.asarray(Wv, np.float32)
    bv = np.asarray(bv, np.float32)
    Wo = np.asarray(Wo, np.float32)
    bo = np.asarray(bo, np.float32)

    from concourse.bass_utils import run_bass_kernel_spmd

    trace = os.environ.get("KERNEL_TRACE", "0") == "1"
    if trace:
        _install_ntff_shim()

    wl_p = _pack_k(Wl, 8)
    bl_p = np.ascontiguousarray(bl.reshape(2, 128).T).astype(np.float32)
    in_maps = []
    for c in range(8):
        b, g = divmod(c, 4)
        sl = slice(256 * g, 256 * g + 256)
        in_maps.append(
            {
                "xT": _pack_k(x[b].T, 8),
                "wq": _pack_k(Wq[:, sl] * SCALE, 8),
                "bq": np.ascontiguousarray((bq[sl] * SCALE).reshape(2, 128).T).astype(np.float32),
                "wl": wl_p,
                "bl": bl_p,
                "wk": _pack_k(Wk[:, sl], 2),
                "wv": _pack_k(Wv[:, sl], 2),
                "wo": _pack_k(Wo[sl, :], 2),
            }
        )

    nc = _get_nc()
    res = run_bass_kernel_spmd(nc, in_maps, core_ids=list(range(8)), trace=trace)
    if trace and res.exec_time_ns is not None:
        print(f"HW exec time: {res.exec_time_ns} ns")
        _STATE["exec_time_ns"] = res.exec_time_ns

    parts = [np.asarray(res.results[c]["out"], np.float32) for c in range(8)]
    const = (bv @ Wo + bo).astype(np.float32)
    out = np.empty((2, 2048, 1024), np.float32)
    for b in range(2):
        out[b] = parts[4 * b] + parts[4 * b + 1] + parts[4 * b + 2] + parts[4 * b + 3] + const
    return out
